# revision 19
# baseline (speedup 1.0000x reference)
"""DiagonalLinear: y = x * w + b (elementwise over features).

x: (16384, 4096) f32, w: (4096,) f32, b: (4096,) f32.

Sharding: data-parallel over the batch dim across 8 NeuronCores (2048 rows
each), weight/bias replicated — fully elementwise, no collectives.

Per-core kernel (Tile framework, one NEFF run SPMD on cores 0-7):
  - w|b packed host-side into one [1, 8192] tensor, partition-broadcast to
    a [128, 8192] SBUF const tile by a single HWDGE DMA; a tiny
    high-priority DVE read absorbs that dependency into the vector
    engine's clock so steady-state TensorTensor ops carry one sync-wait.
  - 8 iterations over [128, 2*4096] tiles (two 128-row blocks per tile,
    4 MiB per DMA): load on the SP HWDGE ring, DVE mul+add in place
    (bit-exact vs the fp32 reference), store on the ACT HWDGE ring.
  - bufs=4 single work pool double-buffers load/compute/store; SBUF use:
    4*32 KiB work + 32 KiB consts per partition.

The kernel is DMA-fabric/DVE co-limited: 64 MiB/core of mandatory HBM
traffic at ~425 GB/s effective (~155 us) and 141 us of DVE fp32
tensor_tensor work overlap to ~180-210 us wall depending on neighbor load
on the shared device.
"""

import numpy as np

import concourse.bacc as bacc
import concourse.mybir as mybir
import concourse.tile as tile
from concourse.bass_utils import run_bass_kernel_spmd

N_CORES = 8
BATCH = 16384
D = 4096
ROWS_PER_CORE = BATCH // N_CORES  # 2048
P = 128

Q = 2      # 128-row blocks per SBUF tile -> 4 MiB DMAs, 8 iterations
BUFS = 4   # work-tile slots: 4*32KiB + 32KiB consts = 160KiB/partition

_CACHE = {}


def build_nc(q=Q, bufs=BUFS):
    nc = bacc.Bacc()
    f32 = mybir.dt.float32
    x = nc.dram_tensor("x", [ROWS_PER_CORE, D], f32, kind="ExternalInput")
    wb_in = nc.dram_tensor("wb", [1, 2 * D], f32, kind="ExternalInput")
    y = nc.dram_tensor("y", [ROWS_PER_CORE, D], f32, kind="ExternalOutput")

    n_tiles = ROWS_PER_CORE // (P * q)
    assert n_tiles * P * q == ROWS_PER_CORE

    # tile n, partition p, free (j, d) <-> row n*(q*P) + j*P + p, col d
    x_r = x.rearrange("(n j p) d -> n p j d", p=P, j=q)
    y_r = y.rearrange("(n j p) d -> n p j d", p=P, j=q)

    with tile.TileContext(nc) as tc:
        with (
            tc.tile_pool(name="consts", bufs=1) as cpool,
            tc.tile_pool(name="work", bufs=bufs) as pool,
        ):
            consts = cpool.tile([P, 2 * D], f32)  # [:, :D]=w, [:, D:]=b
            scratch = cpool.tile([P, 1], f32)
            with tc.high_priority():
                nc.sync.dma_start(
                    consts[:, :].rearrange("p (a d) -> p a d", a=2),
                    wb_in[:, :].rearrange("o (a d) -> o a d", a=2).partition_broadcast(P),
                )
                # absorb the const-DMA dep into DVE's clock
                nc.vector.tensor_copy(scratch[:, :], consts[:, 0:1])

            wt = consts[:, 0:D]
            bt = consts[:, D : 2 * D]
            for i in range(n_tiles):
                t = pool.tile([P, q * D], f32)
                t3 = t[:, :].rearrange("p (j d) -> p j d", j=q)
                nc.sync.dma_start(t3, x_r[i])
                for j in range(q):
                    s = t[:, j * D : (j + 1) * D]
                    nc.vector.tensor_mul(s, s, wt)
                    nc.vector.tensor_add(s, s, bt)
                nc.scalar.dma_start(y_r[i], t3)
    nc.compile()
    return nc


def _get_nc():
    if "nc" not in _CACHE:
        _CACHE["nc"] = build_nc()
    return _CACHE["nc"]


def run(input, weight, bias, nc=None, **spmd_kwargs):
    if nc is None:
        nc = _get_nc()
    x = np.ascontiguousarray(input, dtype=np.float32)
    wb = np.ascontiguousarray(
        np.stack([np.asarray(weight), np.asarray(bias)]).astype(np.float32)
    ).reshape(1, 2 * D)
    in_maps = [
        {"x": x[c * ROWS_PER_CORE : (c + 1) * ROWS_PER_CORE], "wb": wb}
        for c in range(N_CORES)
    ]
    res = run_bass_kernel_spmd(nc, in_maps, core_ids=list(range(N_CORES)), **spmd_kwargs)
    out = np.concatenate([r["y"] for r in res.results], axis=0)
    return out, res


def kernel(input, weight, bias):
    out, _ = run(input, weight, bias)
    return out


# revision 20
# speedup vs baseline: 1.2178x; 1.2178x over previous
"""DiagonalLinear: y = x * w + b (elementwise over features).

x: (16384, 4096) f32, w: (4096,) f32, b: (4096,) f32.

Sharding: data-parallel over the batch dim across 8 NeuronCores (2048 rows
each), weight/bias replicated — fully elementwise, no collectives.

Per-core kernel (Tile framework, one NEFF run SPMD on cores 0-7):
  - w|b packed host-side into one [1, 8192] tensor, fetched by a single
    32 KiB DMA, then broadcast across all 128 partitions OFF the DMA
    fabric: a K=1 fp32 PE matmul against a ones row (bit-exact on TRN2:
    1.0*w) writes PSUM chunks that ACT copies into a [128, 8192] SBUF
    const tile. This keeps the saturated 16-SDMA fabric for x/y traffic
    only (64 MiB/core mandatory vs 68 with a DMA-broadcast) — measured
    ~13 us better median under contention.
  - 8 iterations over [128, 2*4096] tiles (4 MiB per DMA): load on the SP
    HWDGE ring, DVE fp32 mul+add in place (bit-exact vs the reference),
    store on the ACT HWDGE ring; bufs=4 single work pool for overlap.

The kernel is DMA-fabric/DVE co-limited: 64 MiB/core at ~425 GB/s
effective (~155 us) overlapping 141 us of DVE tensor_tensor work;
~183-210 us wall depending on neighbor load on the shared device.
"""

import numpy as np

import concourse.bacc as bacc
import concourse.mybir as mybir
import concourse.tile as tile
from concourse.bass_utils import run_bass_kernel_spmd

N_CORES = 8
BATCH = 16384
D = 4096
ROWS_PER_CORE = BATCH // N_CORES  # 2048
P = 128

Q = 2       # 128-row blocks per SBUF tile -> 4 MiB DMAs, 8 iterations
BUFS = 4    # work-tile slots: 4*32KiB + 32KiB consts = 160KiB/partition
MM_N = 512  # one PSUM bank per broadcast matmul

_CACHE = {}


def build_nc(q=Q, bufs=BUFS):
    nc = bacc.Bacc()
    f32 = mybir.dt.float32
    x = nc.dram_tensor("x", [ROWS_PER_CORE, D], f32, kind="ExternalInput")
    wb_in = nc.dram_tensor("wb", [1, 2 * D], f32, kind="ExternalInput")
    y = nc.dram_tensor("y", [ROWS_PER_CORE, D], f32, kind="ExternalOutput")

    n_tiles = ROWS_PER_CORE // (P * q)
    assert n_tiles * P * q == ROWS_PER_CORE

    # tile n, partition p, free (j, d) <-> row n*(q*P) + j*P + p, col d
    x_r = x.rearrange("(n j p) d -> n p j d", p=P, j=q)
    y_r = y.rearrange("(n j p) d -> n p j d", p=P, j=q)

    with tile.TileContext(nc) as tc:
        with (
            tc.tile_pool(name="consts", bufs=1) as cpool,
            tc.tile_pool(name="work", bufs=bufs) as pool,
            tc.tile_pool(name="psum", bufs=4, space="PSUM") as ppool,
        ):
            consts = cpool.tile([P, 2 * D], f32)  # [:, :D]=w, [:, D:]=b
            wsb = cpool.tile([1, 2 * D], f32)
            ones = cpool.tile([1, P], f32)
            with tc.high_priority():
                nc.scalar.dma_start(wsb[:, :], wb_in[:, :])
                nc.gpsimd.memset(ones[:, :], 1.0)
                for k in range(2 * D // MM_N):
                    pt = ppool.tile([P, MM_N], f32)
                    nc.tensor.matmul(
                        pt[:, :], ones[:, :], wsb[:, k * MM_N : (k + 1) * MM_N],
                        start=True, stop=True,
                    )
                    nc.scalar.copy(consts[:, k * MM_N : (k + 1) * MM_N], pt[:, :])

            wt = consts[:, 0:D]
            bt = consts[:, D : 2 * D]
            for i in range(n_tiles):
                t = pool.tile([P, q * D], f32)
                t3 = t[:, :].rearrange("p (j d) -> p j d", j=q)
                nc.sync.dma_start(t3, x_r[i])
                for j in range(q):
                    s = t[:, j * D : (j + 1) * D]
                    nc.vector.tensor_mul(s, s, wt)
                    nc.vector.tensor_add(s, s, bt)
                nc.scalar.dma_start(y_r[i], t3)
    nc.compile()
    return nc


def _get_nc():
    if "nc" not in _CACHE:
        _CACHE["nc"] = build_nc()
    return _CACHE["nc"]


def run(input, weight, bias, nc=None, **spmd_kwargs):
    if nc is None:
        nc = _get_nc()
    x = np.ascontiguousarray(input, dtype=np.float32)
    wb = np.ascontiguousarray(
        np.stack([np.asarray(weight), np.asarray(bias)]).astype(np.float32)
    ).reshape(1, 2 * D)
    in_maps = [
        {"x": x[c * ROWS_PER_CORE : (c + 1) * ROWS_PER_CORE], "wb": wb}
        for c in range(N_CORES)
    ]
    res = run_bass_kernel_spmd(nc, in_maps, core_ids=list(range(N_CORES)), **spmd_kwargs)
    out = np.concatenate([r["y"] for r in res.results], axis=0)
    return out, res


def kernel(input, weight, bias):
    out, _ = run(input, weight, bias)
    return out
